# revision 33
# baseline (speedup 1.0000x reference)
"""Dual-stream transformer block (nn_Block_73675868995998) on 8 TRN2 NeuronCores.

Sharding: pure data-parallel over batch (B=8 -> one batch element per core).
No collectives. Each core computes the full block for its batch element.

Device layout: everything "transposed" [feature, token] so that LN gamma/beta
fold into the weights on the host, biases become per-partition ACT biases, and
no on-device transposes are needed. Host pre-transposes x/y and re-transposes
the outputs (cheap numpy ops, not on the HW critical path).

Key device tricks:
  - LN over the partition dim via ones-vector matmuls (sum and sum-of-squares);
    rstd = exp(-0.5*ln(D*sumsq - sums^2 + D^2*eps) + ln(D)) on ACT (avoids the
    8-cycle/elem single-partition DVE reciprocal); per-token rows broadcast to
    128 partitions by K=1 ones-matmuls; all-bf16 apply hits the DVE 2x mode.
  - Softmax without max-subtraction (scores are small by construction),
    denominator from an appended ones-column in V (matmul M=65).  Each chain's
    den row is stacked across partitions of a per-qb collector (tiny SBUF->SBUF
    DMAs; engine APs may only base at partitions {0,32,64,96}), so ONE [12,512]
    DVE reciprocal serves all heads of a qb tile.  ctx is written out
    unnormalized (the PE never waits on the reciprocal) and normalized in place
    via GpSimd partition_broadcast + 2x bf16 multiply.
  - Attention emitted as build(x), build(y), finish(x), finish(y) so stream y's
    projection/score matmuls fill the PE while x's softmax normalization and
    output projection dependencies resolve (and vice versa at the tail).
  - All matmuls in bf16 with fp32 PSUM accumulation; fp8 was evaluated and
    rejected (e4m3 MLP alone costs 2.2e-2 relmax vs the 2e-2 gate).
  - Big/weight DMAs ride the Sync HWDGE queue; the tiny denominator-stacking
    DMAs ride the GpSimd SWDGE queue so they never delay weight prefetch.
"""

import numpy as np
import ml_dtypes

import concourse.bass as bass
import concourse.bacc as bacc
import concourse.tile as tile
import concourse.mybir as mybir
from concourse.bass_utils import run_bass_kernel_spmd

P = 128
S = 1024      # sequence length
D = 768       # model dim
KO = D // P   # 6 chunks of model dim
H = 12        # heads
HD = 64       # head dim
MLP = 3072
KOM = MLP // P  # 24 chunks of mlp dim
NB = 512      # free-dim tile (one PSUM bank of fp32)
NQ = S // NB  # 2 query/token column tiles
TB = S // P   # 8 token chunks of 128
EPS = 1e-6

F32 = mybir.dt.float32
BF16 = mybir.dt.bfloat16
AF = mybir.ActivationFunctionType
ALU = mybir.AluOpType

N_CORES = 8
_CACHE = {}


# ----------------------------------------------------------------------------
# device program
# ----------------------------------------------------------------------------

def _emit_ln_pair(nc, mm, st, rows, rowsb, lnb, lnt, sqp, ones_col, ones_all,
                  eps_t, pairs):
    """Transposed layernorm for one or two (src, dst) pairs, chunk-interleaved
    so the second stream's DMA/stats overlap the first's row math.

    rstd = exp(-0.5*ln(var+eps)) on ACT (the 1-partition DVE reciprocal is
    ~8 cycles/elem and stalls everything); row broadcasts on the idle GpSimd;
    all-bf16 apply so the DVE runs in 2x packed mode."""
    epsd2_t, lnd_t, warm_t = eps_t
    # dummy op preloads the Ln ACT table set while stats matmuls run, so the
    # later (serial) rstd chain does not eat the 1.3us ACT_TABLE_LOAD
    nc.scalar.activation(warm_t, warm_t, AF.Ln)
    stps = {}
    for i in range(len(pairs)):
        for qb in range(NQ):
            stps[(i, qb)] = st.tile([P, NB], F32, tag="st", name=f"st{i}{qb}")
    for kc in range(KO):
        for i, (src, dst) in enumerate(pairs):
            for qb in range(NQ):
                cs = slice(qb * NB, (qb + 1) * NB)
                sp = stps[(i, qb)]
                sq = sqp.tile([P, NB], BF16, tag="sq", name="sq")
                nc.scalar.activation(sq, src[:, kc, cs], AF.Square)
                nc.tensor.matmul(sp[0:1, :], ones_col, src[:, kc, cs],
                                 start=(kc == 0), stop=(kc == KO - 1))
                nc.tensor.matmul(sp[32:33, :], ones_col, sq,
                                 start=(kc == 0), stop=(kc == KO - 1))
    # Row math on raw sums: u = D*sumsq - sums^2 = D^2*var, and
    # rstd = exp(-0.5*ln(u + D^2*eps) + ln(D)).  The 1/D folds into the
    # Ln/Exp affine inputs, the square goes to ACT: 1 DVE row op per tile.
    # one batched Ln and one batched Exp (different ACT table sets:
    # interleaving them costs a 1.3us ACT_TABLE_LOAD per switch).
    npq = len(pairs) * NQ
    u_all = rowsb.tile([1, npq, NB], F32, tag="rowv", name="u_all")
    for i in range(len(pairs)):
        for qb in range(NQ):
            sp = stps[(i, qb)]
            j = i * NQ + qb
            m2 = rows.tile([1, NB], F32, tag="row", name="m2")
            nc.scalar.activation(m2, sp[0:1, :], AF.Square)
            nc.vector.scalar_tensor_tensor(u_all[:, j, :], sp[32:33, :],
                                           float(D), m2, ALU.mult, ALU.subtract)
    nc.scalar.activation(u_all, u_all, AF.Ln, bias=epsd2_t)
    rr_all = rowsb.tile([1, npq, NB], BF16, tag="rowv16", name="rr_all")
    nc.scalar.activation(rr_all, u_all, AF.Exp, scale=-0.5, bias=lnd_t)
    rr_rows = {(i, qb): rr_all[:, i * NQ + qb, :]
               for i in range(len(pairs)) for qb in range(NQ)}
    # qb-major allocation order matches the apply loop's consumption order,
    # so lnb slot reuse stays acyclic.
    allbcast = {}
    for qb in range(NQ):
        for i in range(len(pairs)):
            rr = rr_rows[(i, qb)]
            mr = rows.tile([1, NB], BF16, tag="row16", name="mr")
            with nc.allow_low_precision(reason="bf16 mean*rstd row"):
                nc.vector.scalar_tensor_tensor(mr, stps[(i, qb)][0:1, :],
                                               1.0 / D, rr,
                                               ALU.mult, ALU.mult)
            # broadcast rows on the PE (idle during LN) via K=1 ones-matmuls
            rb_ps = mm.tile([P, NB], F32, tag="mm", name="rbps")
            nc.tensor.matmul(rb_ps, ones_all[0:1, :], rr,
                             start=True, stop=True)
            mb_ps = mm.tile([P, NB], F32, tag="mm", name="mbps")
            nc.tensor.matmul(mb_ps, ones_all[0:1, :], mr, start=True, stop=True)
            rb = lnb.tile([P, NB], BF16, tag="lnb", name="rb")
            nc.scalar.activation(rb, rb_ps, AF.Copy)
            mb = lnb.tile([P, NB], BF16, tag="lnb", name="mb")
            nc.scalar.activation(mb, mb_ps, AF.Copy)
            allbcast[(i, qb)] = (rb, mb)
    for qb in range(NQ):
        for i, (src, dst) in enumerate(pairs):
            for kc in range(KO):
                rb, mb = allbcast[(i, qb)]
                cs = slice(qb * NB, (qb + 1) * NB)
                t = lnt.tile([P, NB], BF16, tag="lnt", name="lnt")
                nc.vector.tensor_tensor(t, src[:, kc, cs], rb, ALU.mult)
                nc.vector.tensor_tensor(dst[:, kc, cs], t, mb, ALU.subtract)


def _emit_attn(nc, tc, pools, q_src, kv_src, resid, w_dram, b_sb):
    """One cross-attention: q from q_src, k/v from kv_src, in-place residual
    update of `resid` (all [P, KO, S] layouts).

    The head loop is software-pipelined with LAG chains between the
    scores+exp block and the ctx block, so the PE has score-matmul work to do
    while the ScalarEngine computes the exps of earlier chains.

    Softmax denominators: each chain's den row (psum partition 64) is copied
    into one partition of a per-qb collector (DVE ops may re-base the
    partition window between in and out), so a SINGLE [H, NB] DVE reciprocal
    serves all 12 heads of a qb tile. ctx is written to SBUF unnormalized
    (freeing the psum bank immediately; the PE never waits on the recip) and
    normalized in place afterwards via a GpSimd row-broadcast + 2x bf16 mult.
    """
    LAG = 3
    mm, ctxp, s2p, wA, qk, Vp, Ep, ctxT_pool, (dencol, misc), stg = pools
    aq_d, ak_d, av_d, ao_d = w_dram
    bq_sb, bk_sb, bo_sb, ones_all = b_sb

    wv_sb = wA.tile([P, KO, D], BF16, tag="wA", name="wv")
    nc.sync.dma_start(wv_sb, av_d)
    wq_sb = wA.tile([P, KO, D], BF16, tag="wA", name="wq")
    nc.sync.dma_start(wq_sb, aq_d)
    wk_sb = wA.tile([P, KO, D], BF16, tag="wA", name="wk")
    nc.sync.dma_start(wk_sb, ak_d)

    # ---- V projection: V[tok, d] interleaved with ones columns -------------
    V_sb = Vp.tile([P, TB, H, 65], BF16, tag="V", name="V")
    for tb in range(TB):
        nc.vector.memset(V_sb[:, tb, :, 64:65], 1.0)
    for tb in range(TB):
        for off, w, hs in ((0, NB, slice(0, 8)), (NB, D - NB, slice(8, 12))):
            ps = mm.tile([P, NB], F32, tag="mm", name="vps")
            for kc in range(KO):
                nc.tensor.matmul(ps[:, :w],
                                 kv_src[:, kc, tb * P:(tb + 1) * P],
                                 wv_sb[:, kc, off:off + w],
                                 start=(kc == 0), stop=(kc == KO - 1))
            dst = V_sb[:, tb, hs, 0:64]
            src3 = ps[:, :w].rearrange("p (h d) -> p h d", d=64)
            nc.scalar.activation(dst, src3, AF.Copy)

    ctxT_sb = ctxT_pool.tile([P, KO, S], BF16, tag="ctxT", name="ctxT")
    den_coll = [dencol.tile([H, NB], BF16, tag=f"dcoll{qb}", name="dcoll")
                for qb in range(NQ)]

    def emit_ctx(ch):
        # den row lives on psum partition 64; engine APs may only base at
        # partitions {0,32,64,96}, so the per-head stacking into den_coll
        # goes through a tiny SBUF->SBUF DMA (DMA has no base restriction).
        h, mt, po, cs, qb, E = ch
        ctx_ps = ctxp.tile([P, NB], F32, tag="ctx", name="ctxps")
        for tb in range(TB):
            nc.tensor.matmul(ctx_ps[0:65, :], V_sb[:, tb, h, :],
                             E[:, tb, :],
                             start=(tb == 0), stop=(tb == TB - 1))
        stg_t = stg.tile([P, NB], BF16, tag="dstage", name="dstage")
        with nc.allow_low_precision(reason="bf16 softmax denominator"):
            nc.vector.tensor_copy(stg_t[64:65, :], ctx_ps[64:65, :])
        nc.gpsimd.dma_start(den_coll[qb][h:h + 1, :], stg_t[64:65, :])
        nc.vector.tensor_copy(ctxT_sb[po:po + 64, mt, cs], ctx_ps[0:64, :])

    # ---- per head-pair: project q/k then attend (pipelined) ----------------
    chains = []
    done = []
    for mt in range(KO):
        qt = qk.tile([P, S], BF16, tag="qt", name="qt")
        kt = qk.tile([P, S], BF16, tag="kt", name="kt")
        for qb in range(NQ):
            cs = slice(qb * NB, (qb + 1) * NB)
            psq = mm.tile([P, NB], F32, tag="mm", name="psq")
            psk = mm.tile([P, NB], F32, tag="mm", name="psk")
            for kc in range(KO):
                nc.tensor.matmul(psq, wq_sb[:, kc, mt * P:(mt + 1) * P],
                                 q_src[:, kc, cs],
                                 start=(kc == 0), stop=(kc == KO - 1))
                nc.tensor.matmul(psk, wk_sb[:, kc, mt * P:(mt + 1) * P],
                                 kv_src[:, kc, cs],
                                 start=(kc == 0), stop=(kc == KO - 1))
            nc.vector.tensor_scalar_add(qt[:, cs], psq, bq_sb[:, mt:mt + 1])
            nc.vector.tensor_scalar_add(kt[:, cs], psk, bk_sb[:, mt:mt + 1])
        for hh in range(2):
            h = 2 * mt + hh
            po = hh * 64
            for qb in range(NQ):
                cs = slice(qb * NB, (qb + 1) * NB)
                E = Ep.tile([P, TB, NB], BF16, tag="E", name="E")
                for g in range(TB // 2):
                    sps = s2p.tile([P, 2, NB], F32, tag="s2", name="sps")
                    for j in range(2):
                        tb = 2 * g + j
                        nc.tensor.matmul(sps[:, j, :],
                                         kt[po:po + 64, tb * P:(tb + 1) * P],
                                         qt[po:po + 64, cs],
                                         start=True, stop=True)
                    nc.scalar.activation(E[:, 2 * g:2 * g + 2, :], sps, AF.Exp)
                chains.append((h, mt, po, cs, qb, E))
                done.append((h, mt, po, cs, qb))
                if len(chains) > LAG:
                    emit_ctx(chains.pop(0))
    for ch in chains:
        emit_ctx(ch)

    wo_sb = wA.tile([P, KO, D], BF16, tag="wA", name="wo")
    nc.sync.dma_start(wo_sb, ao_d)
    return (ctxT_sb, den_coll, done, wo_sb)


def _emit_attn_finish(nc, pools, state, resid, b_sb):
    """Deferred softmax normalization + output projection.  Emitted after the
    OTHER stream's projections/chains so their PE work fills the normalize
    latency."""
    mm, ctxp, s2p, wA, qk, Vp, Ep, ctxT_pool, (dencol, misc), stg = pools
    bq_sb, bk_sb, bo_sb, ones_all = b_sb
    ctxT_sb, den_coll, done, wo_sb = state

    rcp_coll = []
    for qb in range(NQ):
        rc = dencol.tile([H, NB], BF16, tag=f"rcoll{qb}", name="rcoll")
        with nc.allow_low_precision(reason="bf16 softmax-denominator recip"):
            nc.vector.reciprocal(rc, den_coll[qb])
        rcp_coll.append(rc)
    for h, mt, po, cs, qb in done:
        # DMA the head's recip row back to partition 0 (legal broadcast
        # source), broadcast to all 128 partitions on GpSimd, then multiply
        # the matching 64-partition window in place (all inputs of a tensor
        # op must share the partition base).
        rst = misc.tile([1, NB], BF16, tag="rst", name="rst")
        nc.sync.dma_start(rst, rcp_coll[qb][h:h + 1, :])
        rbb = misc.tile([P, NB], BF16, tag="rbb", name="rbb")
        nc.gpsimd.partition_broadcast(rbb, rst)
        tgt = ctxT_sb[po:po + 64, mt, cs]
        nc.vector.tensor_tensor(tgt, tgt, rbb[po:po + 64, :], ALU.mult)

    # ---- output projection + in-place residual -----------------------------
    for dm in range(KO):
        for qb in range(NQ):
            cs = slice(qb * NB, (qb + 1) * NB)
            ps = mm.tile([P, NB], F32, tag="mm", name="ops")
            for kc in range(KO):
                nc.tensor.matmul(ps, wo_sb[:, kc, dm * P:(dm + 1) * P],
                                 ctxT_sb[:, kc, cs],
                                 start=(kc == 0), stop=(kc == KO - 1))
            nc.vector.scalar_tensor_tensor(resid[:, dm, cs], ps,
                                           bo_sb[:, dm:dm + 1],
                                           resid[:, dm, cs], ALU.add, ALU.add)


def _emit_mlp(nc, pools, xcf, resid, out_d, w1_d, w2_d, b1_sb, b2_sb):
    """The two qb column-tiles of each mt share one psum tile [P,2,NB], so a
    single Gelu (fc1) / bias+residual+store (fc2) covers both at the same
    per-partition bias."""
    mm, wM, w2p, h1p, stg, warm_t = pools
    nc.scalar.activation(warm_t, warm_t, AF.Gelu)
    h1 = h1p.tile([P, KOM, S], BF16, tag="h1", name="h1")
    for mt in range(KOM):
        w1c = wM.tile([P, KO, P], BF16, tag="w1c", name="w1c")
        nc.sync.dma_start(w1c, w1_d[:, mt])
        ps = mm.tile([P, 2, NB], F32, tag="mm2b", name="f1ps")
        for qb in range(NQ):
            cs = slice(qb * NB, (qb + 1) * NB)
            for kc in range(KO):
                nc.tensor.matmul(ps[:, qb, :], w1c[:, kc, :],
                                 xcf[:, kc, cs],
                                 start=(kc == 0), stop=(kc == KO - 1))
        nc.scalar.activation(h1[:, mt, :].rearrange("p (b n) -> p b n", n=NB),
                             ps, AF.Gelu, bias=b1_sb[:, mt:mt + 1])
    for dm in range(KO):
        w2c = w2p.tile([P, KOM, P], BF16, tag="w2c", name="w2c")
        nc.sync.dma_start(w2c, w2_d[:, dm])
        ps2 = mm.tile([P, 2, NB], F32, tag="mm2b", name="f2ps")
        for qb in range(NQ):
            cs = slice(qb * NB, (qb + 1) * NB)
            for mt in range(KOM):
                nc.tensor.matmul(ps2[:, qb, :], w2c[:, mt, :],
                                 h1[:, mt, cs],
                                 start=(mt == 0), stop=(mt == KOM - 1))
        o = stg.tile([P, 2, NB], F32, tag="stg", name="f2o")
        nc.vector.scalar_tensor_tensor(
            o, ps2, b2_sb[:, dm:dm + 1],
            resid[:, dm, :].rearrange("p (b n) -> p b n", n=NB),
            ALU.add, ALU.add)
        nc.sync.dma_start(out_d[:, dm, :].rearrange("p (b n) -> p b n", n=NB),
                          o)


def build(n_iters=1):
    if n_iters in _CACHE:
        return _CACHE[n_iters]
    nc = bacc.Bacc("TRN2", target_bir_lowering=False, debug=False,
                   enable_asserts=False, num_devices=N_CORES)

    def din(name, shape, dt):
        return nc.dram_tensor(name, shape, dt, kind="ExternalInput").ap()

    def dout(name, shape, dt):
        return nc.dram_tensor(name, shape, dt, kind="ExternalOutput").ap()

    io = {}
    for s in ("x", "y"):
        io[f"{s}T"] = din(f"{s}T", [P, KO, S], BF16)
        for wn in ("aq", "ak", "av", "ao"):
            io[f"{wn}_{s}"] = din(f"{wn}_{s}", [P, KO, D], BF16)
        io[f"a1_{s}"] = din(f"a1_{s}", [P, KOM, KO, P], BF16)
        io[f"a2_{s}"] = din(f"a2_{s}", [P, KO, KOM, P], BF16)
        for bn in ("bq", "bk", "bo", "b2"):
            io[f"{bn}_{s}"] = din(f"{bn}_{s}", [P, KO], F32)
        io[f"b1_{s}"] = din(f"b1_{s}", [P, KOM], F32)
        io[f"o{s}T"] = dout(f"o{s}T", [P, KO, S], F32)

    with tile.TileContext(nc) as tc:
        for _it in range(n_iters):
            _emit_all(tc, nc, io)

    nc.compile()
    _CACHE[n_iters] = nc
    return nc


def _emit_all(tc, nc, io):
        import contextlib
        with contextlib.ExitStack() as ctx:
            mm = ctx.enter_context(tc.tile_pool(name="mm", bufs=2, space="PSUM"))
            const = ctx.enter_context(tc.tile_pool(name="const", bufs=1))
            resid_p = ctx.enter_context(tc.tile_pool(name="resid", bufs=1))
            xc_p = ctx.enter_context(tc.tile_pool(name="xc", bufs=2))
            rows = ctx.enter_context(tc.tile_pool(name="rows", bufs=3))
            lnb = ctx.enter_context(tc.tile_pool(name="lnb", bufs=3))
            rowsb = ctx.enter_context(tc.tile_pool(name="rowsb", bufs=1))
            stg = ctx.enter_context(tc.tile_pool(name="stg", bufs=1))
            # LN scratch hoisted to the whole-body scope: scoping them per
            # phase makes the next phase's pools allocate over their released
            # SBUF zones, serializing attention starts behind the LN apply.
            sqp = ctx.enter_context(tc.tile_pool(name="sq", bufs=2))
            lnt = ctx.enter_context(tc.tile_pool(name="lnt", bufs=1))

            ones_col = const.tile([P, 1], BF16, name="ones_col")
            nc.vector.memset(ones_col, 1.0)
            ones_all = const.tile([P, P], BF16, name="ones_all")
            nc.vector.memset(ones_all, 1.0)
            epsd2_t = const.tile([1, 1], F32, name="epsd2_t")
            nc.vector.memset(epsd2_t, EPS * D * D)
            warm_t = const.tile([1, 1], F32, name="warm_t")
            nc.vector.memset(warm_t, 1.0)
            lnd_t = const.tile([1, 1], F32, name="lnd_t")
            nc.vector.memset(lnd_t, float(np.log(D)))
            eps_t = (epsd2_t, lnd_t, warm_t)

            xT_sb = resid_p.tile([P, KO, S], BF16, tag="xT", name="xT_sb")
            yT_sb = resid_p.tile([P, KO, S], BF16, tag="yT", name="yT_sb")
            nc.sync.dma_start(yT_sb, io["yT"])
            nc.sync.dma_start(xT_sb, io["xT"])

            b_sb = {}
            for s in ("x", "y"):
                for bn, sh in (("bq", [P, KO]), ("bk", [P, KO]), ("bo", [P, KO]),
                               ("b2", [P, KO]), ("b1", [P, KOM])):
                    t = const.tile(sh, F32, name=f"{bn}_{s}_sb")
                    nc.sync.dma_start(t, io[f"{bn}_{s}"])
                    b_sb[f"{bn}_{s}"] = t

            # ---- LN1 -> centered/scaled inputs (bf16) ----------------------
            xc_x = xc_p.tile([P, KO, S], BF16, tag="xc", name="xc_x")
            xc_y = xc_p.tile([P, KO, S], BF16, tag="xc", name="xc_y")
            with tc.tile_pool(name="st1", bufs=4, space="PSUM") as st:
                _emit_ln_pair(nc, mm, st, rows, rowsb, lnb, lnt, sqp, ones_col,
                              ones_all, eps_t,
                              [(yT_sb, xc_y), (xT_sb, xc_x)])

            # ---- attention (both streams) ----------------------------------
            with (
                tc.tile_pool(name="ctxps", bufs=2, space="PSUM") as ctxp,
                tc.tile_pool(name="s2ps", bufs=2, space="PSUM") as s2p,
                tc.tile_pool(name="wA", bufs=4) as wA,
                tc.tile_pool(name="qk", bufs=2) as qk,
                tc.tile_pool(name="Vp", bufs=1) as Vp,
                tc.tile_pool(name="Ep", bufs=4) as Ep,
                tc.tile_pool(name="ctxT", bufs=2) as ctxT_pool,
                tc.tile_pool(name="dencol", bufs=2) as dencol,
                tc.tile_pool(name="amisc", bufs=2) as misc,
            ):
                pools = (mm, ctxp, s2p, wA, qk, Vp, Ep, ctxT_pool,
                         (dencol, misc), stg)
                bx = (b_sb["bq_x"], b_sb["bk_x"], b_sb["bo_x"], ones_all)
                by = (b_sb["bq_y"], b_sb["bk_y"], b_sb["bo_y"], ones_all)
                st_x = _emit_attn(nc, tc, pools, xc_x, xc_y, xT_sb,
                                  (io["aq_x"], io["ak_x"], io["av_x"],
                                   io["ao_x"]), bx)
                st_y = _emit_attn(nc, tc, pools, xc_y, xc_x, yT_sb,
                                  (io["aq_y"], io["ak_y"], io["av_y"],
                                   io["ao_y"]), by)
                _emit_attn_finish(nc, pools, st_x, xT_sb, bx)
                _emit_attn_finish(nc, pools, st_y, yT_sb, by)

            # ---- LN2 + MLP (stream-serial) ---------------------------------
            with (
                tc.tile_pool(name="st2", bufs=2, space="PSUM") as st,
                tc.tile_pool(name="mmx", bufs=2, space="PSUM") as mmx,
                tc.tile_pool(name="wM", bufs=5) as wM,
                tc.tile_pool(name="w2p", bufs=3) as w2p,
                tc.tile_pool(name="h1p", bufs=1) as h1p,
                tc.tile_pool(name="stgo", bufs=2) as stgo,
            ):
                mpools = (mmx, wM, w2p, h1p, stgo, warm_t)
                xcf_x = xc_p.tile([P, KO, S], BF16, tag="xc", name="xcf_x")
                _emit_ln_pair(nc, mm, st, rows, rowsb, lnb, lnt, sqp, ones_col,
                              ones_all, eps_t, [(xT_sb, xcf_x)])
                _emit_mlp(nc, mpools, xcf_x, xT_sb, io["oxT"],
                          io["a1_x"], io["a2_x"], b_sb["b1_x"], b_sb["b2_x"])
                xcf_y = xc_p.tile([P, KO, S], BF16, tag="xc", name="xcf_y")
                _emit_ln_pair(nc, mm, st, rows, rowsb, lnb, lnt, sqp, ones_col,
                              ones_all, eps_t, [(yT_sb, xcf_y)])
                _emit_mlp(nc, mpools, xcf_y, yT_sb, io["oyT"],
                          io["a1_y"], io["a2_y"], b_sb["b1_y"], b_sb["b2_y"])


# ----------------------------------------------------------------------------
# host side
# ----------------------------------------------------------------------------

def _to_pko(w):
    """[Din, M] -> [P, Din//P, M] so that lhsT chunk kc is w[kc*128+p, m]."""
    din, m = w.shape
    return np.ascontiguousarray(
        w.reshape(din // P, P, m).transpose(1, 0, 2))


def _vec_pk(b):
    """[Dout] -> [P, Dout//P] per-partition bias layout."""
    return np.ascontiguousarray(b.reshape(-1, P).T)


def _prep_weights(i):
    """Fold LN gamma/beta + 1/sqrt(HD) into weights, cast to bf16, lay out."""
    f = np.float32
    gx, bx = i["ln_attn_g"].astype(f), i["ln_attn_b"].astype(f)
    gy, by = i["ln_gattn_g"].astype(f), i["ln_gattn_b"].astype(f)
    gfx, bfx = i["ln_ffn_g"].astype(f), i["ln_ffn_b"].astype(f)
    gfy, bfy = i["ln_gffn_g"].astype(f), i["ln_gffn_b"].astype(f)
    sc = np.float32(1.0 / np.sqrt(HD))

    out = {}

    def attn_set(s, wq, bq, wk, bk, wv, bv, wo, bo, gq, betaq, gkv, betakv):
        out[f"aq_{s}"] = (wq * gq[:, None] * sc)
        out[f"bq_{s}"] = ((bq + betaq @ wq) * sc)
        out[f"ak_{s}"] = (wk * gkv[:, None])
        out[f"bk_{s}"] = (bk + betakv @ wk)
        out[f"av_{s}"] = (wv * gkv[:, None])
        out[f"ao_{s}"] = wo
        # V's bias passes through softmax additively (rows sum to 1),
        # so it folds through wo into the output-projection bias.
        out[f"bo_{s}"] = bo + (bv + betakv @ wv) @ wo

    attn_set("x", i["wq"].astype(f), i["bq"].astype(f), i["wk"].astype(f),
             i["bk"].astype(f), i["wv"].astype(f), i["bv"].astype(f),
             i["wo"].astype(f), i["bo"].astype(f), gx, bx, gy, by)
    attn_set("y", i["gwq"].astype(f), i["gbq"].astype(f), i["gwk"].astype(f),
             i["gbk"].astype(f), i["gwv"].astype(f), i["gbv"].astype(f),
             i["gwo"].astype(f), i["gbo"].astype(f), gy, by, gx, bx)

    out["a1_x"] = i["w1"].astype(f) * gfx[:, None]
    out["b1_x"] = i["b1"].astype(f) + bfx @ i["w1"].astype(f)
    out["a2_x"] = i["w2"].astype(f)
    out["b2_x"] = i["b2"].astype(f)
    out["a1_y"] = i["gw1"].astype(f) * gfy[:, None]
    out["b1_y"] = i["gb1"].astype(f) + bfy @ i["gw1"].astype(f)
    out["a2_y"] = i["gw2"].astype(f)
    out["b2_y"] = i["gb2"].astype(f)

    m = {}
    for s in ("x", "y"):
        for wn in ("aq", "ak", "av", "ao"):
            m[f"{wn}_{s}"] = _to_pko(out[f"{wn}_{s}"]).astype(ml_dtypes.bfloat16)
        a1 = _to_pko(out[f"a1_{s}"])  # [P, KO, MLP]
        a1 = a1.reshape(P, KO, KOM, P).transpose(0, 2, 1, 3)
        m[f"a1_{s}"] = np.ascontiguousarray(a1).astype(ml_dtypes.bfloat16)
        a2 = _to_pko(out[f"a2_{s}"])  # [P, KOM, D]
        a2 = a2.reshape(P, KOM, KO, P).transpose(0, 2, 1, 3)
        m[f"a2_{s}"] = np.ascontiguousarray(a2).astype(ml_dtypes.bfloat16)
        for bn in ("bq", "bk", "bo", "b2", "b1"):
            m[f"{bn}_{s}"] = _vec_pk(out[f"{bn}_{s}"]).astype(np.float32)
    return m


def _prep_in_maps(inputs):
    wm = _prep_weights(inputs)
    x = np.asarray(inputs["x"], dtype=np.float32)
    y = np.asarray(inputs["y"], dtype=np.float32)
    in_maps = []
    for c in range(N_CORES):
        im = dict(wm)
        im["xT"] = _to_pko(np.ascontiguousarray(x[c].T)).astype(ml_dtypes.bfloat16)
        im["yT"] = _to_pko(np.ascontiguousarray(y[c].T)).astype(ml_dtypes.bfloat16)
        in_maps.append(im)
    return in_maps


def _post(results, x, y):
    # The device carries the residual stream in bf16; the input's bf16
    # quantization residue is known exactly on the host, so add it back.
    xs, ys = [], []
    for c in range(N_CORES):
        for nm, src_full, acc in (("oxT", x, xs), ("oyT", y, ys)):
            oT = results[c][nm]  # [P, KO, S]
            o = oT.transpose(1, 0, 2).reshape(D, S).T
            sr = src_full[c]
            corr = sr - sr.astype(ml_dtypes.bfloat16).astype(np.float32)
            acc.append(o + corr)
    return (np.ascontiguousarray(np.stack(xs)).astype(np.float32),
            np.ascontiguousarray(np.stack(ys)).astype(np.float32))


def kernel(**inputs):
    nc = build()
    in_maps = _prep_in_maps(inputs)
    res = run_bass_kernel_spmd(nc, in_maps, list(range(N_CORES)))
    x = np.asarray(inputs["x"], dtype=np.float32)
    y = np.asarray(inputs["y"], dtype=np.float32)
    return _post(res.results, x, y)


if __name__ == "__main__":
    # smoke test with random inputs of the right shapes
    rng = np.random.default_rng(0)
    d = {"x": rng.standard_normal((8, S, D), dtype=np.float32),
         "y": rng.standard_normal((8, S, D), dtype=np.float32)}
    for nm in ["wq", "wk", "wv", "wo", "gwq", "gwk", "gwv", "gwo"]:
        d[nm] = (rng.standard_normal((D, D)) * 0.02).astype(np.float32)
    for nm in ["bq", "bk", "bv", "bo", "gbq", "gbk", "gbv", "gbo"]:
        d[nm] = np.zeros(D, np.float32)
    d["w1"] = (rng.standard_normal((D, MLP)) * 0.02).astype(np.float32)
    d["b1"] = np.zeros(MLP, np.float32)
    d["w2"] = (rng.standard_normal((MLP, D)) * 0.02).astype(np.float32)
    d["b2"] = np.zeros(D, np.float32)
    d["gw1"] = (rng.standard_normal((D, MLP)) * 0.02).astype(np.float32)
    d["gb1"] = np.zeros(MLP, np.float32)
    d["gw2"] = (rng.standard_normal((MLP, D)) * 0.02).astype(np.float32)
    d["gb2"] = np.zeros(D, np.float32)
    for nm in ["ln_attn", "ln_gattn", "ln_ffn", "ln_gffn"]:
        d[nm + "_g"] = np.ones(D, np.float32)
        d[nm + "_b"] = np.zeros(D, np.float32)
    o = kernel(**d)
    print("out shapes:", o[0].shape, o[1].shape)

